# revision 64
# baseline (speedup 1.0000x reference)
"""Trainium2 Bass kernel (v19, 150.3us vs 160.2us baseline) for the
attention-scoring module.

    q = query @ Wq.T + bq                               # (B, D)
    ref[b,d,k] = sum_e enc[k,b,e] * Wref[d,e] + bref[d]
    u[b,k] = sum_d v[d] * tanh(ref[b,d,k] + q[b,d])
    out = 10 * tanh(u)                                  # (B, K)

Data-parallel over batch: core c owns b in [32c, 32c+32).

The kernel is scalar(activation)-engine bound: D*K*BL/128 = 131k tanh
cycles @1.2GHz per core is the roofline, and the main ACTIVATE stream now
runs gap-free.  Changes vs the v7 baseline:
  - 14 of 32 rows on single-pass fp8 DoubleRow (hw rel err 1.803e-2,
    matches numpy sim exactly); slabs of 2 b-rows align the 14/18 split.
    (PE runs at 1 output column/cycle regardless of dtype; DR only doubles
    contraction depth per instruction, so multi-pass fp8 schemes lose.)
  - fp8 DR chunks 512 wide (half the matmul instruction count).
  - final logit in 2 chunks (rows 0:96 mid-stream, 96:128 at end); last
    b's combine is split per kp-half with dc order reversed so only one
    TT + 2 strip matmuls gate the last ACTIVATE.  (Merging ALL finals to
    the end regressed 6us: the 254KB out-DMA must stream out mid-kernel.)
  - fp16 q-path (fp32 matmuls ran LOW/HIGH double-pass on the startup
    critical path) emitted between b0's dc groups so the in-order PE queue
    doesn't stall on its constants; dummy tanh preloads the ACT table.
  - b31's 4 output rows skip the u_sb round-trip entirely: tanh runs on
    the strip psum tile and the out-DMA does the partition-stride gather
    (removes a 1.4us swDGE completion wait from the last chain).
  - enc pieces (fp8 and bf16) load both ec-halves in one DMA instruction
    via a DRAM-side rearrange (sync-engine DMA issue is ~0.6us each; the
    early phase was issue-rate bound).
"""

import os
import sys

import numpy as np

os.environ.setdefault("JAX_COMPILATION_CACHE_DIR", "/tmp/jaxcache")

for _p in ("/opt/trn_rl_repo", "/opt/pypackages"):
    if _p not in sys.path:
        sys.path.append(_p)

import ml_dtypes

E = 256
D = 256
K = 2048
B = 256
NCORES = 8
BL = B // NCORES          # 32 batch rows per core
SLAB_B = 2                # b-rows per enc DMA slab
SLAB_N = SLAB_B * K       # 4096
NSLAB = BL // SLAB_B      # 16
# slab type sequence: '8' = fp8 single-pass, 'b' = bf16; 7*2=14 fp8 rows
SLAB_SEQ = ("8", "b", "8", "b", "8", "b", "8", "b",
            "8", "b", "8", "b", "8", "b", "b", "b")
NB8 = SLAB_SEQ.count("8") * SLAB_B     # 14
NBB = BL - NB8                          # 18
N8 = NB8 * K
NB = NBB * K
C_CLIP = 10.0
S_ENC = 16.0
S_W = 512.0
DESCALE = 1.0 / (S_ENC * S_W)

_compiled = None
last_exec_time_ns = None
last_results = None


def _build():
    from concourse import bacc, bass, tile
    from concourse.alu_op_type import AluOpType as ALU

    mybir = bass.mybir
    dt = mybir.dt
    AF = mybir.ActivationFunctionType

    nc = bacc.Bacc("TRN2", target_bir_lowering=False, debug=False,
                   num_devices=NCORES)

    # fp8 enc: rows 0:128 = stream A (e 0:128), 128:256 = B
    enc8_t = nc.declare_dram_parameter("enc8", [E, N8], dt.float8e4, isOutput=False)
    # bf16 enc, two row-halves
    encb_t = nc.declare_dram_parameter("encb", [E, NB], dt.bfloat16, isOutput=False)
    cf32_t = nc.declare_dram_parameter("cf32", [128, 3], dt.float32, isOutput=False)
    cbf16_t = nc.declare_dram_parameter("cbf16", [128, 512], dt.bfloat16, isOutput=False)
    cfp8_t = nc.declare_dram_parameter("cfp8", [128, 512], dt.float8e4, isOutput=False)
    cfp16_t = nc.declare_dram_parameter("cfp16", [128, 577], dt.float16, isOutput=False)
    out_p = nc.declare_dram_parameter("out", [128, 512], dt.float32, isOutput=True)

    with tile.TileContext(nc) as tc:
        with (
            tc.tile_pool(name="const", bufs=1) as constp,
            tc.tile_pool(name="enc", bufs=3) as encp,
            tc.tile_pool(name="tt", bufs=10) as tp,
            tc.tile_pool(name="tail", bufs=2) as tailp,
            tc.tile_pool(name="psum_m", bufs=3, space="PSUM") as pmp,
            tc.tile_pool(name="psum_s", bufs=2, space="PSUM") as psp,
        ):
            # ---- constants: packed DMAs (small/urgent first) ----
            cfp8_sb = constp.tile([128, 512], dt.float8e4)
            cf32_sb = constp.tile([128, 3], dt.float32)
            cbf16_sb = constp.tile([128, 512], dt.bfloat16)
            cfp16_sb = constp.tile([128, 577], dt.float16)
            bias_sb = constp.tile([128, 2 * BL], dt.float32)   # [:, dc*32 + b]
            u_sb = constp.tile([128, 512], dt.float32)         # [b*4+jj, kk]

            # ---- enc slab loading ----
            def alloc_slab8(s):
                return encp.tile([128, 2 * SLAB_N], dt.float8e4, tag="enc8",
                                 name=f"enc8_s{s}")

            def emit_pieces8(t8, s, q0, q1, dep=None):
                # both ec-halves in ONE DMA (sync issue is ~0.6us per instr)
                w = K
                for q in range(q0, q1):
                    dst = t8[:].rearrange("p (two m) -> p two m", two=2)
                    src = enc8_t[0:256,
                                 s * SLAB_N + q * w:s * SLAB_N + (q + 1) * w]
                    ins = nc.sync.dma_start(
                        dst[:, :, q * w:(q + 1) * w],
                        src.rearrange("(two p) n -> p two n", two=2))
                    if dep is not None:
                        tile.add_dep_helper(ins.ins, dep.ins,
                                            reason="defer enc prefetch")

            def alloc_slabb(s):
                return encp.tile([128, 2 * SLAB_N], dt.bfloat16, tag="encb",
                                 name=f"encb_s{s}")

            def emit_piecesb(t, s, q0, q1, dep=None):
                # both ec-halves in ONE DMA, same as the fp8 pieces
                w = K
                for q in range(q0, q1):
                    dst = t[:].rearrange("p (two m) -> p two m", two=2)
                    src = encb_t[0:256,
                                 s * SLAB_N + q * w:s * SLAB_N + (q + 1) * w]
                    ins = nc.sync.dma_start(
                        dst[:, :, q * w:(q + 1) * w],
                        src.rearrange("(two p) n -> p two n", two=2))
                    if dep is not None:
                        tile.add_dep_helper(ins.ins, dep.ins,
                                            reason="defer enc prefetch")

            # per-type dram slab index per seq position
            i8, ib, slab_j = 0, 0, []
            for t in SLAB_SEQ:
                if t == "8":
                    slab_j.append(i8); i8 += 1
                else:
                    slab_j.append(ib); ib += 1

            slab0 = alloc_slab8(slab_j[0])
            # first half-piece (b0's kp0 columns) so matmuls start ASAP
            slab0_2 = slab0[:].rearrange("p (two m) -> p two m", two=2)
            nc.sync.dma_start(
                slab0_2[:, :, 0:1024],
                enc8_t[0:256, 0:1024].rearrange("(two p) n -> p two n", two=2))
            nc.sync.dma_start(cfp8_sb[:], cfp8_t[:])
            nc.sync.dma_start(cfp16_sb[:], cfp16_t[:])
            nc.sync.dma_start(cf32_sb[:], cf32_t[:])
            nc.sync.dma_start(
                slab0_2[:, :, 1024:K],
                enc8_t[0:256, 1024:K].rearrange("(two p) n -> p two n", two=2))

            wq_sb = cfp16_sb[:, 0:512]       # fp16 [:, (ec*2+dc)*128 + d]
            query_sb = cfp16_sb[:, 512:576]  # fp16 [:, ec*32 + b]
            v0_sb = cfp16_sb[:, 576:577]     # fp16 |v| even-rank column
            cbias_sb = cf32_sb[:, 0:2]
            ratio_sb = cf32_sb[:, 2:3]       # v1/v0 per pair-partition
            wref_sb = cbf16_sb[:, 0:512]     # bf16 W [(ec*2+dc)*128 + d]
            w8_sb = cfp8_sb[:, 0:512]        # fp8 W pairs [dc*256 + i*128 + m]

            # ---- q_rawT = (query @ Wq'.T).T per dc-chunk, + (bref' + bq') ----
            # emitted INSIDE the main loop between b0-kp0's dc groups so the
            # in-order PE queue doesn't stall b0's matmuls on the cfp16 DMA
            def emit_qpath():
                for dcq in range(2):
                    qps = psp.tile([128, BL], dt.float32, tag="st")
                    for ec in range(2):
                        nc.tensor.matmul(
                            qps[:],
                            wq_sb[:, (ec * 2 + dcq) * 128:(ec * 2 + dcq + 1) * 128],
                            query_sb[:, ec * BL:(ec + 1) * BL],
                            start=(ec == 0), stop=(ec == 1),
                        )
                    nc.vector.tensor_scalar_add(
                        bias_sb[:, dcq * BL:(dcq + 1) * BL],
                        qps[:], cbias_sb[:, dcq:dcq + 1])
                # bf16 W not needed until si=1 (~20us in); issue its DMA late
                nc.sync.dma_start(cbf16_sb[:], cbf16_t[:])

            t6 = constp.tile([128, 512], dt.float32)
            o6 = constp.tile([128, 512], dt.float32)
            # dummy tanh: preloads the scalar engine's Tanh table off the
            # critical path (first real ACTIVATE skips ACT_TABLE_LOAD).
            # Reads an unwritten tile (garbage in, result discarded) so it
            # has no data dependency and runs right at preamble end.
            nc.scalar.activation(t6[0:32, 0:1], o6[0:32, 0:1], AF.Tanh)

            # ---- per-b tail: combine over both kp halves at once; the LAST
            # b is split per half so little work gates the final chain ----
            def emit_w_cols(w, w1, tts, cols):
                nc.vector.tensor_scalar_mul(w1[:, cols], tts[1][:, cols],
                                            ratio_sb[:, 0:1])
                nc.vector.tensor_add(w[:, cols], w1[:, cols], tts[0][:, cols])

            def emit_w(tts, b):
                # w' = t0 + ratio * t1  (TS 2x + TT 2x), fp16, 2048 wide
                w1 = tp.tile([128, 2048], dt.float16, tag="w1", bufs=2)
                w = tp.tile([128, 2048], dt.float16, tag="w", bufs=2)
                emit_w_cols(w, w1, tts, slice(0, 2048))
                return w

            def emit_strip_mms(st4, w, jjs):
                for jj in jjs:
                    nc.tensor.matmul(
                        st4[32 * jj:32 * jj + 1, :],
                        v0_sb,
                        w[:, jj * 512:(jj + 1) * 512],
                        start=True, stop=True,
                        skip_group_check=True,
                        tile_position=(0, 32 * jj),
                    )

            def emit_strip_out(st4, b):
                sp = tailp.tile([128, 512], dt.float32, tag="sp")
                nc.vector.tensor_copy(sp[:], st4[:])
                # sync-engine (hwDGE) DMA: faster completion-sem than the
                # gpsimd swDGE path (~1.3us) that gated the final ACT
                nc.sync.dma_start(u_sb[4 * b:4 * b + 4, :],
                                  sp[0:128:32, :])

            def emit_strips(st4, w, b):
                emit_strip_mms(st4, w, range(4))
                emit_strip_out(st4, b)

            def emit_final(rows, dma_rows=None):
                nc.scalar.activation(t6[rows, :], u_sb[rows, :], AF.Tanh)
                nc.vector.tensor_scalar_mul(o6[rows, :], t6[rows, :], C_CLIP)
                r = rows if dma_rows is None else dma_rows
                nc.sync.dma_start(out_p[r, :], o6[r, :])

            pend = []
            prev_mm = None

            def tail_pump():
                while len(pend) > 1:
                    emit_strips(*pend.pop(0))

            # ---- unified main loop: fp8 / bf16 slabs interleaved ----
            def alloc_and_prefetch(si, dep):
                typ, j = SLAB_SEQ[si], slab_j[si]
                if typ == "8":
                    t = alloc_slab8(j)
                    emit_pieces8(t, j, 0, SLAB_B, dep=dep)
                else:
                    t = alloc_slabb(j)
                    emit_piecesb(t, j, 0, SLAB_B, dep=dep)
                return t

            cur_slab = slab0
            for si, typ in enumerate(SLAB_SEQ):
                nxt_slab = None
                for b_in in range(SLAB_B):
                    b = SLAB_B * si + b_in
                    last_b = (b == BL - 1)
                    st4 = psp.tile([128, 512], dt.float32, tag="st")
                    btts = [tp.tile([128, 2048], dt.float16, tag="tt", bufs=5,
                                    name=f"tt_{b}_{dcx}")
                            for dcx in range(2)]
                    if typ == "8":
                        enc_pair = cur_slab[:].rearrange("p (two n) -> p two n", two=2)
                    for kp in range(2):
                        if si == 0 and (b_in, kp) == (0, 1):
                            emit_pieces8(cur_slab, slab_j[0], 1, 2, dep=prev_mm)
                        pf_now = ((b_in, kp) == (1, 0)) if si == 0 else \
                                 ((b_in, kp) == (0, 1))
                        if pf_now and si + 1 < NSLAB:
                            nxt_slab = alloc_and_prefetch(si + 1, prev_mm)
                        first_mm = None
                        # for the very last (b, kp): do dc1 first so the
                        # half-combine's TS runs during dc0's ACT and only
                        # the TT gates the final chain
                        for dc in ((1, 0) if (last_b and kp == 1) else (0, 1)):
                            psd = pmp.tile([128, 1024], dt.float32, tag="psd")
                            if typ == "8":
                                for kb in range(2):
                                    nseg = b_in * K + kp * 1024 + kb * 512
                                    ins = nc.tensor.matmul(
                                        psd[:, kb * 512:(kb + 1) * 512],
                                        w8_sb[:, dc * 256:(dc + 1) * 256].rearrange(
                                            "p (two m) -> p two m", two=2),
                                        enc_pair[:, :, nseg:nseg + 512],
                                        start=True, stop=True,
                                        perf_mode=mybir.MatmulPerfMode.DoubleRow,
                                        skip_group_check=True,
                                    )
                                    if first_mm is None:
                                        first_mm = ins
                            else:
                                for ec in range(2):
                                    for kb in range(2):
                                        nseg = b_in * K + kp * 1024 + kb * 512
                                        ins = nc.tensor.matmul(
                                            psd[:, kb * 512:(kb + 1) * 512],
                                            wref_sb[:, (ec * 2 + dc) * 128:(ec * 2 + dc + 1) * 128],
                                            cur_slab[:, ec * SLAB_N + nseg:
                                                     ec * SLAB_N + nseg + 512],
                                            start=(ec == 0), stop=(ec == 1),
                                            skip_group_check=True,
                                        )
                                        if first_mm is None:
                                            first_mm = ins
                            if (si, b_in, kp, dc) == (0, 0, 0, 0):
                                emit_qpath()
                            nc.scalar.activation(
                                btts[dc][:, kp * 1024:(kp + 1) * 1024],
                                psd[:], AF.Tanh,
                                bias=bias_sb[:, dc * BL + b:dc * BL + b + 1],
                                scale=(DESCALE if typ == "8" else 1.0))
                        if kp == 0 and last_b:
                            # flush b30's strips + antepenultimate final now,
                            # then pre-combine b31's kp0 half
                            for args in pend:
                                emit_strips(*args)
                            pend.clear()
                            w31_1 = tp.tile([128, 2048], dt.float16, tag="w1",
                                            bufs=2, name="w1_last")
                            w31 = tp.tile([128, 2048], dt.float16, tag="w",
                                          bufs=2, name="w_last")
                            emit_w_cols(w31, w31_1, btts, slice(0, 1024))
                        if kp == 1:
                            if last_b:
                                # jj0/1 read the kp0 half (already combined).
                                # b24..b30's final (rows 96:124) rides during
                                # b31's chain; b31's own 4 rows skip the u_sb
                                # round-trip: tanh directly on the strip psum,
                                # out-DMA does the partition-stride gather.
                                emit_strip_mms(st4, w31, (0, 1))
                                emit_final(slice(96, 128), slice(96, 124))
                                emit_w_cols(w31, w31_1, btts, slice(1024, 2048))
                                emit_strip_mms(st4, w31, (2, 3))
                                t31 = tailp.tile([128, 512], dt.float32,
                                                 tag="sp")
                                nc.scalar.activation(t31[:], st4[:], AF.Tanh)
                                o31 = tailp.tile([128, 512], dt.float32,
                                                 tag="sp")
                                nc.vector.tensor_scalar_mul(o31[:], t31[:],
                                                            C_CLIP)
                                nc.sync.dma_start(out_p[124:128, :],
                                                  o31[0:128:32, :])
                            else:
                                w = emit_w(btts, b)
                                pend.append((st4, w, b))
                                tail_pump()
                        prev_mm = first_mm
                        # single mid-stream final: b23's strips are emitted
                        # once b24 is pushed (si=12, b_in=0, kp=1)
                        if (si, b_in, kp) == (12, 1, 0):
                            emit_final(slice(0, 96))
                cur_slab = nxt_slab

    nc.compile()
    return nc


def _prep_inputs(encoder_output, query, Wq, bq, Wref, bref, v):
    bf16 = ml_dtypes.bfloat16
    fp16 = np.float16
    e4 = ml_dtypes.float8_e4m3fn if hasattr(ml_dtypes, "float8_e4m3fn") else ml_dtypes.float8_e4m3

    v = np.asarray(v, np.float32)
    sgn = np.where(v >= 0, 1.0, -1.0).astype(np.float32)
    va = np.abs(v)
    order = np.argsort(va, kind="stable")
    c0_idx, c1_idx = order[0::2], order[1::2]
    v0, v1 = va[c0_idx], va[c1_idx]
    ratio = (v1 / v0).astype(np.float32)
    perm = np.concatenate([c0_idx, c1_idx])          # new d order (dc-major)

    Wp = (np.asarray(Wref, np.float32) * sgn[:, None])[perm]     # (256, 256)
    Wqp = (np.asarray(Wq, np.float32) * sgn[:, None])[perm]
    cbias = (np.asarray(bref, np.float32) + np.asarray(bq, np.float32)) * sgn
    cbias = cbias[perm]

    def chunk4(w):                                   # (E, 256d) -> (512, 128)
        return np.ascontiguousarray(
            w.reshape(2, 128, 2, 128).transpose(0, 2, 1, 3).reshape(512, 128))

    def pack(w4):                                    # (4*128, X) -> (128, 4*X)
        x = w4.shape[1]
        return w4.reshape(4, 128, x).transpose(1, 0, 2).reshape(128, 4 * x)

    # bf16 W pack: WT (E, D') where D' columns are [c0 | c1]
    WT = np.ascontiguousarray(Wp.T)                  # (E, 256) cols dc-major
    wref_p = pack(chunk4(WT)).astype(bf16)           # (128, 512)
    wq_p = pack(chunk4(np.ascontiguousarray(Wqp.T))).astype(fp16)  # (128, 512)

    # fp8 W pairs: per dc, lhsT[e, i, m] = Wp[dc*128+m, i*128+e] * S_W
    w8 = np.empty((128, 512), np.float32)
    for dc in range(2):
        chunk = Wp[dc * 128:(dc + 1) * 128] * S_W    # (128 d, 256 e)
        for i in range(2):
            w8[:, dc * 256 + i * 128:dc * 256 + (i + 1) * 128] = \
                chunk[:, i * 128:(i + 1) * 128].T
    w8 = np.clip(w8, -240.0, 240.0).astype(e4)

    cbias_p = cbias.reshape(2, 128).T                # (128, 2)
    ratio_p = ratio.reshape(128, 1)
    v0_p = v0.reshape(128, 1).astype(fp16)
    queryT = np.ascontiguousarray(np.asarray(query, np.float32).T)  # (E, B)

    # row lists by slab type (within each core's 32-row block)
    rows8, rowsb = [], []
    for s, t in enumerate(SLAB_SEQ):
        (rows8 if t == "8" else rowsb).extend(range(SLAB_B * s, SLAB_B * (s + 1)))

    enc = np.asarray(encoder_output, np.float32)     # (K, B, E)
    encT = enc.transpose(2, 1, 0)                    # (E, B, K) view

    in_maps = []
    for c in range(NCORES):
        bs = slice(c * BL, (c + 1) * BL)
        enc_c = encT[:, bs, :]                       # (E, 32, K)
        enc8 = np.ascontiguousarray(enc_c[:, rows8, :]).reshape(E, N8)
        enc8 = np.clip(enc8 * S_ENC, -240.0, 240.0).astype(e4)
        encb = np.ascontiguousarray(enc_c[:, rowsb, :]).reshape(E, NB).astype(bf16)

        q_c = queryT[:, bs]                          # (256, 32)
        q_p = q_c.reshape(2, 128, BL).transpose(1, 0, 2).reshape(128, 2 * BL)
        cf32 = np.ascontiguousarray(
            np.concatenate([cbias_p, ratio_p], axis=1), dtype=np.float32)
        cfp16 = np.ascontiguousarray(np.concatenate(
            [wq_p, q_p.astype(fp16), v0_p], axis=1), dtype=fp16)
        in_maps.append({
            "enc8": enc8,
            "encb": encb,
            "cf32": cf32,
            "cbf16": wref_p,
            "cfp8": w8,
            "cfp16": cfp16,
        })
    return in_maps


def kernel(**inputs):
    global _compiled, last_exec_time_ns, last_results
    from concourse import bass_utils

    if _compiled is None:
        _compiled = _build()
    nc = _compiled

    in_maps = _prep_inputs(**inputs)
    res = bass_utils.run_bass_kernel_spmd(nc, in_maps, core_ids=list(range(NCORES)))
    last_exec_time_ns = res.exec_time_ns
    last_results = res
    out = np.concatenate(
        [r["out"].reshape(BL, K) for r in res.results], axis=0)
    return out
